# revision 32
# baseline (speedup 1.0000x reference)
"""Trainium2 Bass kernel for nn_CTR_Block_77077483094613 (gnn_message_passing).

Strategy (data-parallel over N across 8 cores, 4 samples per core):

Math simplifications applied on host (all exact, verified vs reference):
  * softmax(x1[u]-x2[v], axis=v) is independent of u (x1 cancels), so the
    attention tensor collapses to s2[n,c,v] = softmax(-x2[n,c,v]) and the
    attention einsum collapses to g[n,o,t] = sum_v s2[n,c(o),v]*x3[n,o,t,v]
    broadcast over u.  w1/b1 are unused.
  * A-mix branch re-parameterized: einsum(A, conv4(x)) == conv4(H) + rank-1
    bias, with H = einsum('uv,nctv->nctu', A, x) computed on host (linear
    input transform, im2col-style).  The rank-1 bias b4[o]*rowsum(A)[u] is
    folded in as a 65th input channel of H.
  * All BatchNorms folded into conv weights/biases on host.

Device pipeline per sample:
  conv2+softmax -> s2 ; build s2-scaled conv3 weights (DVE/gpsimd bcast mul)
  g via 25 psum-accumulated strided matmuls (v-slices of x), sample pairs
  row-tiled on the PE (K=64 halves, dual-issued on disjoint row groups) ;
  conv4 on H (K=65) ; yb = relu(y2+g) fused on evac into a t-padded buffer ;
  tcn = 9 shifted-tap matmuls + residual conv accumulated in one psum ;
  final relu(x*1+bias) on evac.

Schedule notes (the perf-critical part):
  * PE warmup on a memset tile starts at ~t=0 (no DMA dependency) so the
    HAM clock-gate opens before real work arrives.
  * Bulk input DMAs (x pair0, h s0/s1, tcn weights) all emitted up-front on
    the Sync queue, BEFORE any output DMA, so outputs never head-of-line
    block inputs.  Pair-1 inputs (x, h s2/s3) prefetch on the GpSimd queue.
  * w3s for pair 0 is built on the DVE (split in two v-halves so the g
    matmuls can start after the first half); pair 1's w3s is built on the
    otherwise-idle GpSimd engine, off the critical path.
  * tcn residual convs for pair 1 are K=64 row-tiles at partitions 0/64 so
    the a/b sample pair dual-issues on disjoint PE row groups.
"""

import numpy as np

N, CIN, COUT, T, V = 32, 64, 128, 256, 25
IC = COUT // 4
EPS = 1e-5
NCORES = 8
NS = N // NCORES          # samples per core
TV = T * V                # 6400
TILE = 500                # free-dim tile: 20 t positions x 25 u
PAD = 4 * V               # 100

_CACHE = {}


def _patch_tile_drain():
    """walrus in this container allows only 1 sync-wait per CTRL inst; split
    the TileContext end-of-kernel drain accordingly."""
    import concourse.tile as tile
    from concourse import mybir
    from concourse.vector_clock import ScopedClock

    if getattr(tile.TileContext, "_drain_split_patched", False):
        return

    def _drain_and_barrier(self, tick_clock, wait_clock):
        drain_inst = self.nc.sync.drain()
        wait_clock.add_sem_waits(
            drain_inst.ins, ScopedClock({None: tick_clock.global_clock})
        )
        si = drain_inst.ins.sync_info
        waits = list(si.on_wait or [])
        if len(waits) > 1:
            si.on_wait = waits[:1]
            for w in waits[1:]:
                d2 = self.nc.sync.drain()
                d2.ins.sync_info = mybir.SyncInfo(on_wait=[w], on_update=[])
        self.nc.all_engine_barrier()
        assert self.sems is not None
        popped = self.nc._tile_sem_poison_stack.pop()
        assert popped is self._sem_poison
        self.nc.clear_and_free_semaphores(list(self.sems.allocated().values()))
        self.nc.all_engine_barrier()

    tile.TileContext._drain_and_barrier = _drain_and_barrier
    tile.TileContext._drain_split_patched = True


def _split_multi_waits(nc, mybir):
    """walrus here allows only 1 sync-wait per instruction: hoist extra waits
    onto same-engine NoOps inserted just before the instruction."""
    k = 0
    for fn in nc.m.functions:
        for bb in fn.blocks:
            insts = bb.instructions
            i = 0
            while i < len(insts):
                ins = insts[i]
                si = ins.sync_info
                waits = list(si.on_wait) if si and si.on_wait else []
                if len(waits) > 1:
                    si.on_wait = waits[:1]
                    for w in waits[1:]:
                        nop = mybir.InstNoOp(
                            name=f"wsplit-{k}",
                            engine=ins.engine,
                            ins=[],
                            outs=[],
                            sync_info=mybir.SyncInfo(on_wait=[w], on_update=[]),
                        )
                        k += 1
                        insts.insert(i, nop)
                        i += 1
                i += 1


def _build_nc():
    from contextlib import ExitStack

    import concourse.bass as bass
    import concourse.tile as tile
    from concourse.tile_rust import add_dep_helper
    from concourse import mybir

    _patch_tile_drain()
    f32 = mybir.dt.float32
    bf16 = mybir.dt.bfloat16

    nc = bass.Bass()

    # ---- DRAM parameters (per-core shapes) ----
    # small consts packed into two blobs so the prologue pays 2 DMA-issue
    # latencies instead of 9 (each dma_start costs ~620ns on its engine)
    d_x = nc.declare_dram_parameter("x", [NS, CIN, TV], bf16, isOutput=False)
    d_h = nc.declare_dram_parameter("h", [NS, CIN + 1, TV], bf16, isOutput=False)
    d_xm = nc.declare_dram_parameter("xm", [2 * CIN, (NS // 2) * V], bf16, isOutput=False)
    # cf32 cols: w2t2 0:64 | w3t2 64:192 | gbias 192 | bout 193 | b2p 194
    d_cf32 = nc.declare_dram_parameter("cf32", [2 * CIN, 195], f32, isOutput=False)
    # cb16 cols: w4t 0:128 | wrt2 128:256 | sel2 256:384 | w3t2 384:512
    d_cb16 = nc.declare_dram_parameter("cb16", [2 * CIN, 576], bf16, isOutput=False)
    d_wtt = nc.declare_dram_parameter("wtt", [COUT, 9 * COUT], bf16, isOutput=False)
    d_out = nc.declare_dram_parameter("out", [NS, COUT, TV], bf16, isOutput=True)

    # tile widths: 12 x 500 + 1 x 400 = 6400
    widths = [TILE] * 12 + [400]
    offs = np.cumsum([0] + widths).tolist()

    with tile.TileContext(nc) as tc, ExitStack() as ctx:
        const = ctx.enter_context(tc.tile_pool(name="const", bufs=1))
        xpool = ctx.enter_context(tc.tile_pool(name="xpair", bufs=2))
        hpool = ctx.enter_context(tc.tile_pool(name="htile", bufs=3))
        ybpool = ctx.enter_context(tc.tile_pool(name="yb", bufs=3))
        spool = ctx.enter_context(tc.tile_pool(name="small", bufs=2))
        w3spool = ctx.enter_context(tc.tile_pool(name="w3s", bufs=2))
        opool = ctx.enter_context(tc.tile_pool(name="otile", bufs=6))
        pg = ctx.enter_context(tc.tile_pool(name="pg", bufs=2, space="PSUM"))
        py = ctx.enter_context(tc.tile_pool(name="py", bufs=2, space="PSUM"))
        po = ctx.enter_context(tc.tile_pool(name="po", bufs=4, space="PSUM"))

        # ---- tiny per-pair inputs on the GpSimd DMA queue (never blocked
        # behind bulk transfers); both pairs in one DMA ----
        xmall = spool.tile([2 * CIN, (NS // 2) * V], bf16, tag="xm")
        nc.sync.dma_start(xmall[:], d_xm[:])
        xm2s = [xmall[:, pair * V:(pair + 1) * V] for pair in range(NS // 2)]

        # ---- const blobs + bulk pair-0 inputs on the Sync queue, in
        # need-order, before any output DMA ----
        cf32 = const.tile([2 * CIN, 195], f32)
        nc.sync.dma_start(cf32[:], d_cf32[:])
        w2t2 = cf32[:, 0:2 * IC]
        w3t2 = cf32[:, 2 * IC:2 * IC + COUT]
        gbias = cf32[:, 192:193]
        bout = cf32[:, 193:194]
        b2p = cf32[0:2 * IC, 194:195]
        cb16 = const.tile([2 * CIN, 576], bf16)
        nc.sync.dma_start(cb16[:], d_cb16[:])
        w4t = cb16[0:CIN + 1, 0:COUT]
        w3t2b = cb16[:, 3 * COUT:4 * COUT]
        w2t2b = cb16[:, 4 * COUT:4 * COUT + 2 * IC]
        wrt2a = cb16[0:CIN, COUT:2 * COUT]
        wrt2b = cb16[CIN:2 * CIN, COUT:2 * COUT]
        sel2 = cb16[0:2, 2 * COUT:3 * COUT]

        # ---- PE warmup on a memset tile: opens the HAM clock-gate from
        # ~t=0 with no DMA dependency.  Warmup matmuls are interleaved with
        # the prologue matmuls (PE executes in order, so a block of warmups
        # ahead of px2 would delay the whole softmax->w3s chain) ----
        warm = const.tile([COUT, 512], bf16)
        nc.vector.memset(warm[:], 0.0)
        zeros = const.tile([COUT, TILE], f32)
        nc.vector.memset(zeros[:], 0.0)
        tbl = spool.tile([1, 1], f32, tag="tbl")
        nc.vector.memset(tbl[:], 0.0)
        nc.scalar.activation(tbl[:], tbl[:],
                             mybir.ActivationFunctionType.Exp)

        def warmup(k):
            for i in range(k):
                wps = po.tile([COUT, 512], f32, tag="pot")
                nc.tensor.matmul(wps[:], warm[:, 0:COUT], warm[:])

        # ---- per-pair prologue, staged so both pairs' chains interleave:
        # conv2 -> softmax -> replicate s2 across partitions -> build the
        # s2-scaled conv3 weights ----
        def prologue_sm(pair):
            # conv2 (PE) + softmax (ACT/DVE) + partition-collapse (gpsimd DMA)
            px2 = py.tile([2 * IC, V], f32, tag="pyt")
            nc.tensor.matmul(px2[:], w2t2b, xm2s[pair])
            e2 = spool.tile([2 * IC, V], f32, tag="e2")
            # exp(-(w2@xm + b2)) = Exp(in*-1 + (-b2)); b2p holds -b2
            nc.scalar.activation(
                e2[:], px2[:], mybir.ActivationFunctionType.Exp,
                bias=b2p, scale=-1.0,
            )
            ssum = spool.tile([2 * IC, 1], f32, tag="ssum")
            nc.vector.tensor_reduce(
                ssum[:], e2[:], mybir.AxisListType.X, mybir.AluOpType.add
            )
            rinv = spool.tile([2 * IC, 1], f32, tag="rinv")
            nc.vector.reciprocal(rinv[:], ssum[:])
            s2 = spool.tile([2 * IC, V], bf16, tag="s2")
            nc.vector.tensor_scalar_mul(s2[:], e2[:], rinv[:, 0:1])
            s2rowpair = spool.tile([2, IC * V], bf16, tag="s2row")
            eng = nc.sync if pair == 0 else nc.gpsimd
            c1 = eng.dma_start(s2rowpair[0:1, :], s2[0:IC, :])
            c2 = eng.dma_start(s2rowpair[1:2, :], s2[IC:, :])
            return s2rowpair, (c1, c2)

        def prologue_rep(s2rowpair):
            # replicate to 128 partitions: K=2 matmul against a 0/1
            # selection matrix; evac copies on the otherwise-idle ACT
            # the rhs view transposes (c,v) -> (v,c) so s2rep comes out
            # (v,c)-major: the broadcast multiply then reads a contiguous
            # innermost c dim (uneven split keeps slices at v boundaries)
            s2rep = spool.tile([2 * CIN, IC * V], bf16, tag="s2rep")
            s2r_vc = s2rowpair[:].rearrange("h (c v) -> h v c", c=IC)
            for v0, v1 in ((0, 12), (12, V)):
                o0h, o1h = v0 * IC, v1 * IC
                ps = pg.tile([2 * CIN, 416], f32, tag="pg")
                w = o1h - o0h
                nc.tensor.matmul(ps[:, 0:w], sel2, s2r_vc[:, v0:v1])
                nc.scalar.copy(s2rep[:, o0h:o1h], ps[:, 0:w])
            return s2rep

        def prologue_tt(s2rep, split):
            # W3S[p, (v, m, c)] = w3t2[p, (m, c)] * s2[c, v]
            w3s = w3spool.tile([2 * CIN, V * COUT], bf16, tag="w3s")
            wv = w3s[:].rearrange("p (v m c) -> p v m c", v=V, m=4)
            i0 = w3t2b.rearrange("p (m c) -> p m c", m=4).unsqueeze(1)
            i1 = s2rep[:].rearrange("p (v c) -> p v c", c=IC).unsqueeze(2)
            if split:
                # thirds: DVE is ~2x faster than GpSimd on this op, and the
                # g matmuls consume w3s in v order, so DVE takes the front
                plan = ((nc.vector, (0, 10)), (nc.vector, (10, 20)),
                        (nc.gpsimd, (20, V)))
            else:
                plan = ((nc.gpsimd, (0, V)),)
            for eng, (v0, v1) in plan:
                eng.tensor_tensor(
                    wv[:, v0:v1],
                    i0.broadcast_to([2 * CIN, v1 - v0, 4, IC]),
                    i1[:, v0:v1].broadcast_to([2 * CIN, v1 - v0, 4, IC]),
                    mybir.AluOpType.mult,
                )
            return w3s

        g_last = [None]

        warmup(2)
        sr0, colls0 = prologue_sm(0)
        sr1, colls1 = prologue_sm(1)
        # bulk pair-0 inputs issue on the Sync queue AFTER the pair-0
        # collapse DMAs (same-queue FIFO: the tiny transfers land first
        # instead of starving behind bulk), serialized x2t -> h0 -> wtt -> h1
        x2t_p0 = xpool.tile([2 * CIN, TV], bf16, tag="x2t")
        dma_x2t0 = nc.sync.dma_start(x2t_p0[:], d_x[0:2])
        ht_s0 = hpool.tile([CIN + 1, TV], bf16, tag="ht")
        dma_h0 = nc.sync.dma_start(ht_s0[:], d_h[0])
        add_dep_helper(dma_h0.ins, dma_x2t0.ins, sync=True,
                       reason="serialize bulk inputs: h0 after x2t")
        wtt = const.tile([COUT, 9 * COUT], bf16)
        dma_wtt = nc.sync.dma_start(wtt[:], d_wtt[:])
        add_dep_helper(dma_wtt.ins, dma_h0.ins, sync=True,
                       reason="serialize bulk inputs: wtt after h0")
        ht_s1 = hpool.tile([CIN + 1, TV], bf16, tag="ht")
        dma_h1 = nc.sync.dma_start(ht_s1[:], d_h[1])
        add_dep_helper(dma_h1.ins, dma_wtt.ins, sync=True,
                       reason="serialize bulk inputs: h1 after wtt")
        warmup(2)
        s2rep0 = prologue_rep(sr0)
        warmup(2)
        w3s_p0 = prologue_tt(s2rep0, split=True)
        warmup(8)

        # ---- pair-1 bulk inputs prefetch on the GpSimd queue, serialized
        # after pair-0 bulk so they never congest the critical window ----
        x2t_p1 = xpool.tile([2 * CIN, TV], bf16, tag="x2t")
        dma_x2t1 = nc.gpsimd.dma_start(x2t_p1[:], d_x[2:4])
        add_dep_helper(dma_x2t1.ins, dma_h1.ins, sync=True,
                       reason="serialize bulk inputs: x2t_p1 after h1")
        ht_s2 = hpool.tile([CIN + 1, TV], bf16, tag="ht")
        dma_h2 = nc.gpsimd.dma_start(ht_s2[:], d_h[2])
        add_dep_helper(dma_h2.ins, dma_x2t1.ins, sync=True,
                       reason="serialize bulk inputs: h2 after x2t_p1")

        def g_pair(w3s, x2t):
            # g: 25 accumulated strided matmuls per sample, the two samples
            # row-tiled on disjoint PE row groups (dual-issue)
            pga = pg.tile([COUT, T], f32, tag="pg")
            pgb = pg.tile([COUT, T], f32, tag="pg")
            for v in range(V):
                nc.tensor.matmul(pga[:], w3s[0:CIN, v * COUT:(v + 1) * COUT],
                                 x2t[0:CIN, v::25], start=(v == 0), stop=(v == V - 1))
                gmm = nc.tensor.matmul(pgb[:], w3s[CIN:, v * COUT:(v + 1) * COUT],
                                 x2t[CIN:, v::25], start=(v == 0), stop=(v == V - 1))
            g_last[0] = gmm
            g_a = spool.tile([COUT, T], f32, tag="g_a")
            g_b = spool.tile([COUT, T], f32, tag="g_b")
            nc.scalar.activation(g_a[:], pga[:], mybir.ActivationFunctionType.Identity,
                                 bias=gbias, scale=1.0)
            nc.scalar.activation(g_b[:], pgb[:], mybir.ActivationFunctionType.Identity,
                                 bias=gbias, scale=1.0)
            return g_a, g_b

        def conv4(ht, gq):
            # yb = relu(conv4(H) + g), assembled into a t-padded buffer
            yb = ybpool.tile([COUT, TV + 2 * PAD], bf16, tag="yb")
            nc.vector.memset(yb[:, 0:PAD], 0.0)
            nc.vector.memset(yb[:, PAD + TV:], 0.0)
            for j, (o0, w) in enumerate(zip(offs[:-1], widths)):
                pyt = py.tile([COUT, TILE], f32, tag="pyt")
                nc.tensor.matmul(pyt[:, 0:w], w4t, ht[:, o0:o0 + w])
                t0, tw = o0 // V, w // V
                gview = gq[:, t0:t0 + tw].unsqueeze(2).broadcast_to([COUT, tw, V])
                dst = yb[:, PAD + o0:PAD + o0 + w]
                nc.vector.scalar_tensor_tensor(
                    dst.rearrange("p (t v) -> p t v", v=V),
                    pyt[:, 0:w].rearrange("p (t v) -> p t v", v=V),
                    0.0, gview,
                    mybir.AluOpType.bypass, mybir.AluOpType.add,
                )
                # relu over stt-tile PAIRS: halves the yb writer count, so
                # the downstream tap matmuls carry half the sem-wait edges
                if j % 2 == 1 or j == len(widths) - 1:
                    r0 = offs[j - 1] if j % 2 == 1 else offs[j]
                    rdst = yb[:, PAD + r0:PAD + o0 + w]
                    nc.scalar.activation(rdst, rdst,
                                         mybir.ActivationFunctionType.Relu)
            return yb

        def tcn_evac(pot, w, n, o0, j):
            # final relu(acc + bout) on ACT: keeps the DVE free for the
            # yb-evac stt chain that gates the next sample's taps
            ot = opool.tile([COUT, TILE], bf16, tag="ot")
            nc.scalar.activation(
                ot[:, 0:w], pot[:, 0:w],
                mybir.ActivationFunctionType.Relu,
                bias=bout, scale=1.0)
            nc.sync.dma_start(d_out[n][:, o0:o0 + w], ot[:, 0:w])

        def tcn_single(yb, xrow, wrrow, n):
            # tcn: 9 shifted-tap matmuls + residual conv in one psum
            for j, (o0, w) in enumerate(zip(offs[:-1], widths)):
                pot = po.tile([COUT, TILE], f32, tag="pot")
                rmm = nc.tensor.matmul(pot[:, 0:w], wrrow, xrow[:, o0:o0 + w],
                                 start=True, stop=False)
                add_dep_helper(rmm.ins, g_last[0].ins, sync=False,
                               reason="keep residual MMs behind g in PE stream")
                for k in range(9):
                    nc.tensor.matmul(
                        pot[:, 0:w], wtt[:, k * COUT:(k + 1) * COUT],
                        yb[:, o0 + k * V:o0 + k * V + w],
                        start=False, stop=(k == 8))
                tcn_evac(pot, w, n, o0, j)

        def tcn_paired(yb_a, yb_b, x2t, na):
            # both samples per tile: the two K=64 residual convs dual-issue
            # on disjoint row groups; tap matmuls share weights
            for j, (o0, w) in enumerate(zip(offs[:-1], widths)):
                pot_a = po.tile([COUT, TILE], f32, tag="pot")
                pot_b = po.tile([COUT, TILE], f32, tag="pot")
                rmma = nc.tensor.matmul(pot_a[:, 0:w], wrt2a, x2t[0:CIN, o0:o0 + w],
                                 start=True, stop=False)
                rmmb = nc.tensor.matmul(pot_b[:, 0:w], wrt2b, x2t[CIN:, o0:o0 + w],
                                 start=True, stop=False)
                add_dep_helper(rmma.ins, g_last[0].ins, sync=False,
                               reason="keep residual MMs behind g in PE stream")
                add_dep_helper(rmmb.ins, g_last[0].ins, sync=False,
                               reason="keep residual MMs behind g in PE stream")
                for k in range(9):
                    lhs = wtt[:, k * COUT:(k + 1) * COUT]
                    nc.tensor.matmul(pot_a[:, 0:w], lhs,
                                     yb_a[:, o0 + k * V:o0 + k * V + w],
                                     start=False, stop=(k == 8))
                    nc.tensor.matmul(pot_b[:, 0:w], lhs,
                                     yb_b[:, o0 + k * V:o0 + k * V + w],
                                     start=False, stop=(k == 8))
                tcn_evac(pot_a, w, na, o0, j)
                tcn_evac(pot_b, w, na + 1, o0, j + 1)

        # ---- pair 0: latency-optimized (per-sample tcn starts as soon as
        # the first conv4 evacs land) ----
        g_a0, g_b0 = g_pair(w3s_p0, x2t_p0)
        yb_s0 = conv4(ht_s0, g_a0)
        tcn_single(yb_s0, x2t_p0[0:CIN, :], wrt2a, 0)
        s2rep1 = prologue_rep(sr1)
        w3s_p1 = prologue_tt(s2rep1, split=False)
        yb_s1 = conv4(ht_s1, g_b0)
        tcn_single(yb_s1, x2t_p0[CIN:, :], wrt2b, 1)

        ht_s3 = hpool.tile([CIN + 1, TV], bf16, tag="ht")
        dma_h3 = nc.gpsimd.dma_start(ht_s3[:], d_h[3])
        add_dep_helper(dma_h3.ins, dma_h2.ins, sync=True,
                       reason="serialize bulk inputs: h3 after h2")

        # ---- pair 1: throughput-optimized (paired tcn) ----
        g_a1, g_b1 = g_pair(w3s_p1, x2t_p1)
        yb_s2 = conv4(ht_s2, g_a1)
        yb_s3 = conv4(ht_s3, g_b1)
        tcn_paired(yb_s2, yb_s3, x2t_p1, 2)

    _split_multi_waits(nc, mybir)
    return nc


def _host_prep(inputs):
    x = np.ascontiguousarray(inputs["x"], dtype=np.float32)
    A = np.asarray(inputs["A"], dtype=np.float32)

    s1 = inputs["bn1_g"] / np.sqrt(inputs["bn1_v"] + EPS)
    t1 = inputs["bn1_b"] - inputs["bn1_m"] * s1
    s2n = inputs["bn2_g"] / np.sqrt(inputs["bn2_v"] + EPS)
    t2n = inputs["bn2_b"] - inputs["bn2_m"] * s2n
    sr = inputs["bnr_g"] / np.sqrt(inputs["bnr_v"] + EPS)
    tr = inputs["bnr_b"] - inputs["bnr_m"] * sr

    w2t = np.asarray(inputs["w2"], np.float32).T                     # [64, 32]
    w2t2 = np.zeros((2 * CIN, 2 * IC), np.float32)
    w2t2[0:CIN, 0:IC] = w2t
    w2t2[CIN:, IC:] = w2t
    b2p = np.concatenate([-inputs["b2"], -inputs["b2"]]).astype(np.float32)[:, None]

    w3p = (inputs["w3"] * s1[:, None]).astype(np.float32)            # [128, 64]
    w3t2 = np.concatenate([w3p.T, w3p.T], axis=0).astype(np.float32)  # [128, 128]
    gbias = (s1 * inputs["b3"] + t1).astype(np.float32)[:, None]

    w4p = (inputs["w4"] * s1[:, None]).astype(np.float32)
    w4t = np.zeros((CIN + 1, COUT), np.float32)
    w4t[0:CIN, :] = w4p.T
    w4t[CIN, :] = s1 * inputs["b4"]

    wrp = (inputs["wr"] * sr[:, None]).astype(np.float32)
    wrt2 = np.concatenate([wrp.T, wrp.T], axis=0).astype(np.float32)

    wtp = (inputs["wt"][..., 0] * s2n[:, None, None]).astype(np.float32)  # [128,128,9]
    wtt = np.concatenate([wtp[:, :, k].T for k in range(9)], axis=1)
    wtt = np.ascontiguousarray(wtt, np.float32)                       # [128, 9*128]

    bout = (inputs["bt"] * s2n + t2n + inputs["br"] * sr + tr).astype(np.float32)[:, None]

    # H with rank-1 bias channel: h[n, 0:64, t, u] = sum_v A[u,v] x[n,:,t,v]
    # h[n, 64, t, u] = rowsum(A)[u]
    xf = x.reshape(N * CIN * T, V)
    H = (xf @ A.T).reshape(N, CIN, T, V)
    rA = A.sum(axis=1).astype(np.float32)
    h = np.empty((N, CIN + 1, T * V), np.float32)
    h[:, 0:CIN, :] = H.reshape(N, CIN, T * V)
    h[:, CIN, :] = np.tile(rA, T)[None, :]

    xm = x.mean(axis=2).astype(np.float32)                            # [N, 64, 25]

    sel2 = np.zeros((2, COUT), np.float32)
    sel2[0, 0:CIN] = 1.0
    sel2[1, CIN:] = 1.0

    import ml_dtypes
    bf = ml_dtypes.bfloat16

    # pack small consts into two blobs (one DMA issue each on device)
    cf32 = np.zeros((2 * CIN, 195), np.float32)
    cf32[:, 0:2 * IC] = w2t2
    cf32[:, 2 * IC:2 * IC + COUT] = w3t2
    cf32[:, 192:193] = gbias
    cf32[:, 193:194] = bout
    cf32[0:2 * IC, 194:195] = b2p
    cb16 = np.zeros((2 * CIN, 576), np.float32)
    cb16[0:CIN + 1, 0:COUT] = w4t
    cb16[:, COUT:2 * COUT] = wrt2
    cb16[0:2, 2 * COUT:3 * COUT] = sel2
    cb16[:, 3 * COUT:4 * COUT] = w3t2
    cb16[:, 4 * COUT:4 * COUT + 2 * IC] = w2t2

    consts = dict(cf32=cf32, cb16=cb16.astype(bf), wtt=wtt.astype(bf))
    return x.astype(bf), h.astype(bf), xm, consts


def kernel(**inputs):
    from concourse.bass_utils import run_bass_kernel_spmd

    x, h, xm, consts = _host_prep(inputs)

    if "nc" not in _CACHE:
        _CACHE["nc"] = _build_nc()
    nc = _CACHE["nc"]

    in_maps = []
    for core in range(NCORES):
        sl = slice(core * NS, (core + 1) * NS)
        m = dict(consts)
        m["x"] = np.ascontiguousarray(x[sl].reshape(NS, CIN, TV))
        m["h"] = np.ascontiguousarray(h[sl])
        xmc = xm[sl]                                   # [NS, CIN, V]
        xmblob = np.concatenate(
            [xmc.reshape(NS // 2, 2 * CIN, V)[p] for p in range(NS // 2)],
            axis=1,
        )                                              # [2*CIN, (NS//2)*V]
        m["xm"] = np.ascontiguousarray(xmblob.astype(x.dtype))
        in_maps.append(m)

    res = run_bass_kernel_spmd(nc, in_maps, list(range(NCORES)))
    out = np.concatenate([r["out"] for r in res.results], axis=0)
    return np.ascontiguousarray(out.reshape(N, COUT, T, V), dtype=np.float32)


# revision 33
# speedup vs baseline: 1.0488x; 1.0488x over previous
"""Trainium2 Bass kernel for nn_CTR_Block_77077483094613 (gnn_message_passing).

Strategy (data-parallel over N across 8 cores, 4 samples per core):

Math simplifications applied on host (all exact, verified vs reference):
  * softmax(x1[u]-x2[v], axis=v) is independent of u (x1 cancels), so the
    attention tensor collapses to s2[n,c,v] = softmax(-x2[n,c,v]) and the
    attention einsum collapses to g[n,o,t] = sum_v s2[n,c(o),v]*x3[n,o,t,v]
    broadcast over u.  w1/b1 are unused.
  * A-mix branch re-parameterized: einsum(A, conv4(x)) == conv4(H) + rank-1
    bias, with H = einsum('uv,nctv->nctu', A, x) computed on host (linear
    input transform, im2col-style).  The rank-1 bias b4[o]*rowsum(A)[u] is
    folded in as a 65th input channel of H.
  * All BatchNorms folded into conv weights/biases on host.

Device pipeline per sample:
  conv2+softmax -> s2 ; build s2-scaled conv3 weights (DVE/gpsimd bcast mul)
  g via 25 psum-accumulated strided matmuls (v-slices of x), sample pairs
  row-tiled on the PE (K=64 halves, dual-issued on disjoint row groups) ;
  conv4 on H (K=65) ; yb = relu(y2+g) fused on evac into a t-padded buffer ;
  tcn = 9 shifted-tap matmuls + residual conv accumulated in one psum ;
  final relu(x*1+bias) on evac.

Schedule notes (the perf-critical part):
  * PE warmup on a memset tile starts at ~t=0 (no DMA dependency) so the
    HAM clock-gate opens before real work arrives.
  * Bulk input DMAs (x pair0, h s0/s1, tcn weights) all emitted up-front on
    the Sync queue, BEFORE any output DMA, so outputs never head-of-line
    block inputs.  Pair-1 inputs (x, h s2/s3) prefetch on the GpSimd queue.
  * w3s for pair 0 is built on the DVE (split in two v-halves so the g
    matmuls can start after the first half); pair 1's w3s is built on the
    otherwise-idle GpSimd engine, off the critical path.
  * tcn residual convs for pair 1 are K=64 row-tiles at partitions 0/64 so
    the a/b sample pair dual-issues on disjoint PE row groups.
"""

import numpy as np

N, CIN, COUT, T, V = 32, 64, 128, 256, 25
IC = COUT // 4
EPS = 1e-5
NCORES = 8
NS = N // NCORES          # samples per core
TV = T * V                # 6400
TILE = 500                # free-dim tile: 20 t positions x 25 u
PAD = 4 * V               # 100

_CACHE = {}


def _patch_tile_drain():
    """walrus in this container allows only 1 sync-wait per CTRL inst; split
    the TileContext end-of-kernel drain accordingly."""
    import concourse.tile as tile
    from concourse import mybir
    from concourse.vector_clock import ScopedClock

    if getattr(tile.TileContext, "_drain_split_patched", False):
        return

    def _drain_and_barrier(self, tick_clock, wait_clock):
        drain_inst = self.nc.sync.drain()
        wait_clock.add_sem_waits(
            drain_inst.ins, ScopedClock({None: tick_clock.global_clock})
        )
        si = drain_inst.ins.sync_info
        waits = list(si.on_wait or [])
        if len(waits) > 1:
            si.on_wait = waits[:1]
            for w in waits[1:]:
                d2 = self.nc.sync.drain()
                d2.ins.sync_info = mybir.SyncInfo(on_wait=[w], on_update=[])
        self.nc.all_engine_barrier()
        assert self.sems is not None
        popped = self.nc._tile_sem_poison_stack.pop()
        assert popped is self._sem_poison
        self.nc.clear_and_free_semaphores(list(self.sems.allocated().values()))
        self.nc.all_engine_barrier()

    tile.TileContext._drain_and_barrier = _drain_and_barrier
    tile.TileContext._drain_split_patched = True


def _split_multi_waits(nc, mybir):
    """walrus here allows only 1 sync-wait per instruction: hoist extra waits
    onto same-engine NoOps inserted just before the instruction."""
    k = 0
    for fn in nc.m.functions:
        for bb in fn.blocks:
            insts = bb.instructions
            i = 0
            while i < len(insts):
                ins = insts[i]
                si = ins.sync_info
                waits = list(si.on_wait) if si and si.on_wait else []
                if len(waits) > 1:
                    si.on_wait = waits[:1]
                    for w in waits[1:]:
                        nop = mybir.InstNoOp(
                            name=f"wsplit-{k}",
                            engine=ins.engine,
                            ins=[],
                            outs=[],
                            sync_info=mybir.SyncInfo(on_wait=[w], on_update=[]),
                        )
                        k += 1
                        insts.insert(i, nop)
                        i += 1
                i += 1


def _build_nc():
    from contextlib import ExitStack

    import concourse.bass as bass
    import concourse.tile as tile
    from concourse.tile_rust import add_dep_helper
    from concourse import mybir

    _patch_tile_drain()
    f32 = mybir.dt.float32
    bf16 = mybir.dt.bfloat16

    nc = bass.Bass()

    # ---- DRAM parameters (per-core shapes) ----
    # small consts packed into two blobs so the prologue pays 2 DMA-issue
    # latencies instead of 9 (each dma_start costs ~620ns on its engine)
    d_x = nc.declare_dram_parameter("x", [NS, CIN, TV], bf16, isOutput=False)
    d_h = nc.declare_dram_parameter("h", [NS, CIN + 1, TV], bf16, isOutput=False)
    d_xm = nc.declare_dram_parameter("xm", [2 * CIN, (NS // 2) * V], bf16, isOutput=False)
    # cf32 cols: w2t2 0:64 | w3t2 64:192 | gbias 192 | bout 193 | b2p 194
    d_cf32 = nc.declare_dram_parameter("cf32", [2 * CIN, 195], f32, isOutput=False)
    # cb16 cols: w4t 0:128 | wrt2 128:256 | sel2 256:384 | w3t2 384:512
    d_cb16 = nc.declare_dram_parameter("cb16", [2 * CIN, 576], bf16, isOutput=False)
    d_wtt = nc.declare_dram_parameter("wtt", [COUT, 9 * COUT], bf16, isOutput=False)
    d_out = nc.declare_dram_parameter("out", [NS, COUT, TV], bf16, isOutput=True)

    # tile widths: 12 x 500 + 1 x 400 = 6400
    widths = [TILE] * 12 + [400]
    offs = np.cumsum([0] + widths).tolist()

    with tile.TileContext(nc) as tc, ExitStack() as ctx:
        const = ctx.enter_context(tc.tile_pool(name="const", bufs=1))
        xpool = ctx.enter_context(tc.tile_pool(name="xpair", bufs=2))
        hpool = ctx.enter_context(tc.tile_pool(name="htile", bufs=3))
        ybpool = ctx.enter_context(tc.tile_pool(name="yb", bufs=3))
        spool = ctx.enter_context(tc.tile_pool(name="small", bufs=2))
        w3spool = ctx.enter_context(tc.tile_pool(name="w3s", bufs=2))
        opool = ctx.enter_context(tc.tile_pool(name="otile", bufs=6))
        pg = ctx.enter_context(tc.tile_pool(name="pg", bufs=2, space="PSUM"))
        py = ctx.enter_context(tc.tile_pool(name="py", bufs=2, space="PSUM"))
        po = ctx.enter_context(tc.tile_pool(name="po", bufs=4, space="PSUM"))

        # ---- tiny per-pair inputs on the GpSimd DMA queue (never blocked
        # behind bulk transfers); both pairs in one DMA ----
        xmall = spool.tile([2 * CIN, (NS // 2) * V], bf16, tag="xm")
        nc.sync.dma_start(xmall[:], d_xm[:])
        xm2s = [xmall[:, pair * V:(pair + 1) * V] for pair in range(NS // 2)]

        # ---- const blobs + bulk pair-0 inputs on the Sync queue, in
        # need-order, before any output DMA ----
        cf32 = const.tile([2 * CIN, 195], f32)
        nc.sync.dma_start(cf32[:], d_cf32[:])
        w2t2 = cf32[:, 0:2 * IC]
        w3t2 = cf32[:, 2 * IC:2 * IC + COUT]
        gbias = cf32[:, 192:193]
        bout = cf32[:, 193:194]
        b2p = cf32[0:2 * IC, 194:195]
        cb16 = const.tile([2 * CIN, 576], bf16)
        nc.sync.dma_start(cb16[:], d_cb16[:])
        w4t = cb16[0:CIN + 1, 0:COUT]
        w3t2b = cb16[:, 3 * COUT:4 * COUT]
        w2t2b = cb16[:, 4 * COUT:4 * COUT + 2 * IC]
        wrt2a = cb16[0:CIN, COUT:2 * COUT]
        wrt2b = cb16[CIN:2 * CIN, COUT:2 * COUT]
        sel2 = cb16[0:2, 2 * COUT:3 * COUT]

        # ---- PE warmup on a memset tile: opens the HAM clock-gate from
        # ~t=0 with no DMA dependency.  Warmup matmuls are interleaved with
        # the prologue matmuls (PE executes in order, so a block of warmups
        # ahead of px2 would delay the whole softmax->w3s chain) ----
        warm = const.tile([COUT, 512], bf16)
        nc.vector.memset(warm[:], 0.0)
        zeros = const.tile([COUT, TILE], f32)
        nc.vector.memset(zeros[:], 0.0)
        tbl = spool.tile([1, 1], f32, tag="tbl")
        nc.vector.memset(tbl[:], 0.0)
        nc.scalar.activation(tbl[:], tbl[:],
                             mybir.ActivationFunctionType.Exp)

        def warmup(k):
            for i in range(k):
                wps = po.tile([COUT, 512], f32, tag="pot")
                nc.tensor.matmul(wps[:], warm[:, 0:COUT], warm[:])

        # ---- per-pair prologue, staged so both pairs' chains interleave:
        # conv2 -> softmax -> replicate s2 across partitions -> build the
        # s2-scaled conv3 weights ----
        def prologue_sm(pair):
            # conv2 (PE) + softmax (ACT/DVE) + partition-collapse (gpsimd DMA)
            px2 = py.tile([2 * IC, V], f32, tag="pyt")
            nc.tensor.matmul(px2[:], w2t2b, xm2s[pair])
            e2 = spool.tile([2 * IC, V], f32, tag="e2")
            # exp(-(w2@xm + b2)) = Exp(in*-1 + (-b2)); b2p holds -b2
            nc.scalar.activation(
                e2[:], px2[:], mybir.ActivationFunctionType.Exp,
                bias=b2p, scale=-1.0,
            )
            ssum = spool.tile([2 * IC, 1], f32, tag="ssum")
            nc.vector.tensor_reduce(
                ssum[:], e2[:], mybir.AxisListType.X, mybir.AluOpType.add
            )
            rinv = spool.tile([2 * IC, 1], f32, tag="rinv")
            nc.vector.reciprocal(rinv[:], ssum[:])
            s2 = spool.tile([2 * IC, V], bf16, tag="s2")
            nc.vector.tensor_scalar_mul(s2[:], e2[:], rinv[:, 0:1])
            s2rowpair = spool.tile([2, IC * V], bf16, tag="s2row")
            eng = nc.sync if pair == 0 else nc.gpsimd
            c1 = eng.dma_start(s2rowpair[0:1, :], s2[0:IC, :])
            c2 = eng.dma_start(s2rowpair[1:2, :], s2[IC:, :])
            return s2rowpair, (c1, c2)

        def prologue_rep(s2rowpair):
            # replicate to 128 partitions: K=2 matmul against a 0/1
            # selection matrix; evac copies on the otherwise-idle ACT
            # the rhs view transposes (c,v) -> (v,c) so s2rep comes out
            # (v,c)-major: the broadcast multiply then reads a contiguous
            # innermost c dim (uneven split keeps slices at v boundaries)
            s2rep = spool.tile([2 * CIN, IC * V], bf16, tag="s2rep")
            s2r_vc = s2rowpair[:].rearrange("h (c v) -> h v c", c=IC)
            for v0, v1 in ((0, 12), (12, V)):
                o0h, o1h = v0 * IC, v1 * IC
                ps = pg.tile([2 * CIN, 416], f32, tag="pg")
                w = o1h - o0h
                nc.tensor.matmul(ps[:, 0:w], sel2, s2r_vc[:, v0:v1])
                nc.scalar.copy(s2rep[:, o0h:o1h], ps[:, 0:w])
            return s2rep

        def prologue_tt(s2rep, split):
            # W3S[p, (v, m, c)] = w3t2[p, (m, c)] * s2[c, v]
            w3s = w3spool.tile([2 * CIN, V * COUT], bf16, tag="w3s")
            wv = w3s[:].rearrange("p (v m c) -> p v m c", v=V, m=4)
            i0 = w3t2b.rearrange("p (m c) -> p m c", m=4).unsqueeze(1)
            i1 = s2rep[:].rearrange("p (v c) -> p v c", c=IC).unsqueeze(2)
            if split:
                # thirds: DVE is ~2x faster than GpSimd on this op, and the
                # g matmuls consume w3s in v order, so DVE takes the front
                plan = ((nc.vector, (0, 10)), (nc.vector, (10, 20)),
                        (nc.gpsimd, (20, V)))
            else:
                plan = ((nc.gpsimd, (0, V)),)
            for eng, (v0, v1) in plan:
                eng.tensor_tensor(
                    wv[:, v0:v1],
                    i0.broadcast_to([2 * CIN, v1 - v0, 4, IC]),
                    i1[:, v0:v1].broadcast_to([2 * CIN, v1 - v0, 4, IC]),
                    mybir.AluOpType.mult,
                )
            return w3s

        g_last = [None]

        warmup(2)
        sr0, colls0 = prologue_sm(0)
        sr1, colls1 = prologue_sm(1)
        # bulk pair-0 inputs issue on the Sync queue AFTER the pair-0
        # collapse DMAs (same-queue FIFO: the tiny transfers land first
        # instead of starving behind bulk), serialized x2t -> h0 -> wtt -> h1
        x2t_p0 = xpool.tile([2 * CIN, TV], bf16, tag="x2t")
        dma_x2t0 = nc.sync.dma_start(x2t_p0[:], d_x[0:2])
        ht_s0 = hpool.tile([CIN + 1, TV], bf16, tag="ht")
        dma_h0 = nc.sync.dma_start(ht_s0[:], d_h[0])
        add_dep_helper(dma_h0.ins, dma_x2t0.ins, sync=True,
                       reason="serialize bulk inputs: h0 after x2t")
        wtt = const.tile([COUT, 9 * COUT], bf16)
        dma_wtt = nc.sync.dma_start(wtt[:], d_wtt[:])
        add_dep_helper(dma_wtt.ins, dma_h0.ins, sync=True,
                       reason="serialize bulk inputs: wtt after h0")
        ht_s1 = hpool.tile([CIN + 1, TV], bf16, tag="ht")
        dma_h1 = nc.sync.dma_start(ht_s1[:], d_h[1])
        add_dep_helper(dma_h1.ins, dma_wtt.ins, sync=True,
                       reason="serialize bulk inputs: h1 after wtt")
        warmup(2)
        s2rep0 = prologue_rep(sr0)
        warmup(2)
        w3s_p0 = prologue_tt(s2rep0, split=True)
        warmup(8)

        # ---- pair-1 bulk inputs prefetch on the GpSimd queue, serialized
        # after pair-0 bulk so they never congest the critical window ----
        x2t_p1 = xpool.tile([2 * CIN, TV], bf16, tag="x2t")
        dma_x2t1 = nc.gpsimd.dma_start(x2t_p1[:], d_x[2:4])
        add_dep_helper(dma_x2t1.ins, dma_h1.ins, sync=True,
                       reason="serialize bulk inputs: x2t_p1 after h1")
        ht_s2 = hpool.tile([CIN + 1, TV], bf16, tag="ht")
        dma_h2 = nc.gpsimd.dma_start(ht_s2[:], d_h[2])
        add_dep_helper(dma_h2.ins, dma_x2t1.ins, sync=True,
                       reason="serialize bulk inputs: h2 after x2t_p1")

        def g_pair(w3s, x2t):
            # g: 25 accumulated strided matmuls per sample, the two samples
            # row-tiled on disjoint PE row groups (dual-issue)
            pga = pg.tile([COUT, T], f32, tag="pg")
            pgb = pg.tile([COUT, T], f32, tag="pg")
            for v in range(V):
                nc.tensor.matmul(pga[:], w3s[0:CIN, v * COUT:(v + 1) * COUT],
                                 x2t[0:CIN, v::25], start=(v == 0), stop=(v == V - 1))
                gmm = nc.tensor.matmul(pgb[:], w3s[CIN:, v * COUT:(v + 1) * COUT],
                                 x2t[CIN:, v::25], start=(v == 0), stop=(v == V - 1))
            g_last[0] = gmm
            g_a = spool.tile([COUT, T], f32, tag="g_a")
            g_b = spool.tile([COUT, T], f32, tag="g_b")
            nc.scalar.activation(g_a[:], pga[:], mybir.ActivationFunctionType.Identity,
                                 bias=gbias, scale=1.0)
            nc.scalar.activation(g_b[:], pgb[:], mybir.ActivationFunctionType.Identity,
                                 bias=gbias, scale=1.0)
            return g_a, g_b

        def conv4(ht, gq):
            # yb = relu(conv4(H) + g), assembled into a t-padded buffer
            yb = ybpool.tile([COUT, TV + 2 * PAD], bf16, tag="yb")
            nc.vector.memset(yb[:, 0:PAD], 0.0)
            nc.vector.memset(yb[:, PAD + TV:], 0.0)
            for o0, w in zip(offs[:-1], widths):
                pyt = py.tile([COUT, TILE], f32, tag="pyt")
                nc.tensor.matmul(pyt[:, 0:w], w4t, ht[:, o0:o0 + w])
                t0, tw = o0 // V, w // V
                gview = gq[:, t0:t0 + tw].unsqueeze(2).broadcast_to([COUT, tw, V])
                dst = yb[:, PAD + o0:PAD + o0 + w]
                nc.vector.scalar_tensor_tensor(
                    dst.rearrange("p (t v) -> p t v", v=V),
                    pyt[:, 0:w].rearrange("p (t v) -> p t v", v=V),
                    0.0, gview,
                    mybir.AluOpType.bypass, mybir.AluOpType.add,
                )
                nc.scalar.activation(dst, dst, mybir.ActivationFunctionType.Relu)
            return yb

        def tcn_evac(pot, w, n, o0, j):
            # final relu(acc + bout) on ACT: keeps the DVE free for the
            # yb-evac stt chain that gates the next sample's taps
            ot = opool.tile([COUT, TILE], bf16, tag="ot")
            nc.scalar.activation(
                ot[:, 0:w], pot[:, 0:w],
                mybir.ActivationFunctionType.Relu,
                bias=bout, scale=1.0)
            nc.sync.dma_start(d_out[n][:, o0:o0 + w], ot[:, 0:w])

        def tcn_single(yb, xrow, wrrow, n):
            # tcn: 9 shifted-tap matmuls + residual conv in one psum
            for j, (o0, w) in enumerate(zip(offs[:-1], widths)):
                pot = po.tile([COUT, TILE], f32, tag="pot")
                rmm = nc.tensor.matmul(pot[:, 0:w], wrrow, xrow[:, o0:o0 + w],
                                 start=True, stop=False)
                add_dep_helper(rmm.ins, g_last[0].ins, sync=False,
                               reason="keep residual MMs behind g in PE stream")
                for k in range(9):
                    nc.tensor.matmul(
                        pot[:, 0:w], wtt[:, k * COUT:(k + 1) * COUT],
                        yb[:, o0 + k * V:o0 + k * V + w],
                        start=False, stop=(k == 8))
                tcn_evac(pot, w, n, o0, j)

        def tcn_paired(yb_a, yb_b, x2t, na):
            # both samples per tile: the two K=64 residual convs dual-issue
            # on disjoint row groups; tap matmuls share weights
            for j, (o0, w) in enumerate(zip(offs[:-1], widths)):
                pot_a = po.tile([COUT, TILE], f32, tag="pot")
                pot_b = po.tile([COUT, TILE], f32, tag="pot")
                rmma = nc.tensor.matmul(pot_a[:, 0:w], wrt2a, x2t[0:CIN, o0:o0 + w],
                                 start=True, stop=False)
                rmmb = nc.tensor.matmul(pot_b[:, 0:w], wrt2b, x2t[CIN:, o0:o0 + w],
                                 start=True, stop=False)
                add_dep_helper(rmma.ins, g_last[0].ins, sync=False,
                               reason="keep residual MMs behind g in PE stream")
                add_dep_helper(rmmb.ins, g_last[0].ins, sync=False,
                               reason="keep residual MMs behind g in PE stream")
                for k in range(9):
                    lhs = wtt[:, k * COUT:(k + 1) * COUT]
                    nc.tensor.matmul(pot_a[:, 0:w], lhs,
                                     yb_a[:, o0 + k * V:o0 + k * V + w],
                                     start=False, stop=(k == 8))
                    nc.tensor.matmul(pot_b[:, 0:w], lhs,
                                     yb_b[:, o0 + k * V:o0 + k * V + w],
                                     start=False, stop=(k == 8))
                tcn_evac(pot_a, w, na, o0, j)
                tcn_evac(pot_b, w, na + 1, o0, j + 1)

        # ---- pair 0: latency-optimized (per-sample tcn starts as soon as
        # the first conv4 evacs land) ----
        g_a0, g_b0 = g_pair(w3s_p0, x2t_p0)
        yb_s0 = conv4(ht_s0, g_a0)
        tcn_single(yb_s0, x2t_p0[0:CIN, :], wrt2a, 0)
        s2rep1 = prologue_rep(sr1)
        w3s_p1 = prologue_tt(s2rep1, split=False)
        yb_s1 = conv4(ht_s1, g_b0)
        tcn_single(yb_s1, x2t_p0[CIN:, :], wrt2b, 1)

        ht_s3 = hpool.tile([CIN + 1, TV], bf16, tag="ht")
        dma_h3 = nc.gpsimd.dma_start(ht_s3[:], d_h[3])
        add_dep_helper(dma_h3.ins, dma_h2.ins, sync=True,
                       reason="serialize bulk inputs: h3 after h2")

        # ---- pair 1: throughput-optimized (paired tcn) ----
        g_a1, g_b1 = g_pair(w3s_p1, x2t_p1)
        yb_s2 = conv4(ht_s2, g_a1)
        yb_s3 = conv4(ht_s3, g_b1)
        tcn_paired(yb_s2, yb_s3, x2t_p1, 2)

    _split_multi_waits(nc, mybir)
    return nc


def _host_prep(inputs):
    x = np.ascontiguousarray(inputs["x"], dtype=np.float32)
    A = np.asarray(inputs["A"], dtype=np.float32)

    s1 = inputs["bn1_g"] / np.sqrt(inputs["bn1_v"] + EPS)
    t1 = inputs["bn1_b"] - inputs["bn1_m"] * s1
    s2n = inputs["bn2_g"] / np.sqrt(inputs["bn2_v"] + EPS)
    t2n = inputs["bn2_b"] - inputs["bn2_m"] * s2n
    sr = inputs["bnr_g"] / np.sqrt(inputs["bnr_v"] + EPS)
    tr = inputs["bnr_b"] - inputs["bnr_m"] * sr

    w2t = np.asarray(inputs["w2"], np.float32).T                     # [64, 32]
    w2t2 = np.zeros((2 * CIN, 2 * IC), np.float32)
    w2t2[0:CIN, 0:IC] = w2t
    w2t2[CIN:, IC:] = w2t
    b2p = np.concatenate([-inputs["b2"], -inputs["b2"]]).astype(np.float32)[:, None]

    w3p = (inputs["w3"] * s1[:, None]).astype(np.float32)            # [128, 64]
    w3t2 = np.concatenate([w3p.T, w3p.T], axis=0).astype(np.float32)  # [128, 128]
    gbias = (s1 * inputs["b3"] + t1).astype(np.float32)[:, None]

    w4p = (inputs["w4"] * s1[:, None]).astype(np.float32)
    w4t = np.zeros((CIN + 1, COUT), np.float32)
    w4t[0:CIN, :] = w4p.T
    w4t[CIN, :] = s1 * inputs["b4"]

    wrp = (inputs["wr"] * sr[:, None]).astype(np.float32)
    wrt2 = np.concatenate([wrp.T, wrp.T], axis=0).astype(np.float32)

    wtp = (inputs["wt"][..., 0] * s2n[:, None, None]).astype(np.float32)  # [128,128,9]
    wtt = np.concatenate([wtp[:, :, k].T for k in range(9)], axis=1)
    wtt = np.ascontiguousarray(wtt, np.float32)                       # [128, 9*128]

    bout = (inputs["bt"] * s2n + t2n + inputs["br"] * sr + tr).astype(np.float32)[:, None]

    # H with rank-1 bias channel: h[n, 0:64, t, u] = sum_v A[u,v] x[n,:,t,v]
    # h[n, 64, t, u] = rowsum(A)[u]
    xf = x.reshape(N * CIN * T, V)
    H = (xf @ A.T).reshape(N, CIN, T, V)
    rA = A.sum(axis=1).astype(np.float32)
    h = np.empty((N, CIN + 1, T * V), np.float32)
    h[:, 0:CIN, :] = H.reshape(N, CIN, T * V)
    h[:, CIN, :] = np.tile(rA, T)[None, :]

    xm = x.mean(axis=2).astype(np.float32)                            # [N, 64, 25]

    sel2 = np.zeros((2, COUT), np.float32)
    sel2[0, 0:CIN] = 1.0
    sel2[1, CIN:] = 1.0

    import ml_dtypes
    bf = ml_dtypes.bfloat16

    # pack small consts into two blobs (one DMA issue each on device)
    cf32 = np.zeros((2 * CIN, 195), np.float32)
    cf32[:, 0:2 * IC] = w2t2
    cf32[:, 2 * IC:2 * IC + COUT] = w3t2
    cf32[:, 192:193] = gbias
    cf32[:, 193:194] = bout
    cf32[0:2 * IC, 194:195] = b2p
    cb16 = np.zeros((2 * CIN, 576), np.float32)
    cb16[0:CIN + 1, 0:COUT] = w4t
    cb16[:, COUT:2 * COUT] = wrt2
    cb16[0:2, 2 * COUT:3 * COUT] = sel2
    cb16[:, 3 * COUT:4 * COUT] = w3t2
    cb16[:, 4 * COUT:4 * COUT + 2 * IC] = w2t2

    consts = dict(cf32=cf32, cb16=cb16.astype(bf), wtt=wtt.astype(bf))
    return x.astype(bf), h.astype(bf), xm, consts


def kernel(**inputs):
    from concourse.bass_utils import run_bass_kernel_spmd

    x, h, xm, consts = _host_prep(inputs)

    if "nc" not in _CACHE:
        _CACHE["nc"] = _build_nc()
    nc = _CACHE["nc"]

    in_maps = []
    for core in range(NCORES):
        sl = slice(core * NS, (core + 1) * NS)
        m = dict(consts)
        m["x"] = np.ascontiguousarray(x[sl].reshape(NS, CIN, TV))
        m["h"] = np.ascontiguousarray(h[sl])
        xmc = xm[sl]                                   # [NS, CIN, V]
        xmblob = np.concatenate(
            [xmc.reshape(NS // 2, 2 * CIN, V)[p] for p in range(NS // 2)],
            axis=1,
        )                                              # [2*CIN, (NS//2)*V]
        m["xm"] = np.ascontiguousarray(xmblob.astype(x.dtype))
        in_maps.append(m)

    res = run_bass_kernel_spmd(nc, in_maps, list(range(NCORES)))
    out = np.concatenate([r["out"] for r in res.results], axis=0)
    return np.ascontiguousarray(out.reshape(N, COUT, T, V), dtype=np.float32)
